# revision 59
# baseline (speedup 1.0000x reference)
"""Causal self-attention with RoPE on 8 Trainium2 NeuronCores — v4.

Sharding: tensor-parallel over heads (2 heads/core). Per batch, an
AllToAll redistributes y^T (plus softmax denominators, riding as a 65th
row per head) from head-shards to 256-token shards; each core then
normalizes and projects its tokens with the full W_proj.

v4 (vs v3 baseline at ~219us/rep measured, sim 229us; v4 measures
~124us/rep, sim marginal 141us):
- Per-batch AllToAlls: batch 0's collective is issued mid-rep (hidden
  under batch-1 compute); only batch 1's ~28us rides the rep boundary,
  and its result is consumed two reps later (carry), giving it a full
  rep of slack.
- Queue isolation: collectives + staging/projection DMAs live on the
  Pool (gpsimd) queue; xt prefetch + rope-permutation DMAs on SP; the
  v-transposes on ACT; exp owns ACT otherwise; copies own DVE. The
  causal diagonal mask is a DVE multiply with a precomputed triangular
  constant instead of gpsimd affine_select, so attention never waits on
  the Pool queue.
- xt slab prefetch: all four strip-slab DMA triggers are issued at body
  start so rep i+1's loads run during rep i's attention tail.
- Software pipelining: batch-1's QKV matmuls are interleaved into
  batch-0's (ACT-bound) attention strips, and the carried projections
  into batch-1's strips (late strips carry more work — they have the
  most exp-bound PE slack), keeping the PE stream dense.
"""

import math

import numpy as np
import ml_dtypes

import concourse.bass as bass
import concourse.mybir as mybir
import concourse.tile as tile
from concourse import bacc
from concourse.bass_utils import run_bass_kernel_spmd

B, T, D = 2, 2048, 1024
H, DH = 16, 64
ROPE_BASE = 10000.0
N_CORES = 8
P = 128
TOK = B * T
TOK_PER_CORE = TOK // N_CORES          # 512
DC = D // P                            # 8 contraction chunks
NS = T // 512                          # 4 strips per batch

FP32 = mybir.dt.float32
FP32R = mybir.dt.float32r
BF16 = mybir.dt.bfloat16
AF = mybir.ActivationFunctionType
ALU = mybir.AluOpType

# When True, pre-fill PSUM gap columns so CoreSim's uninitialized-read
# check passes; the values are fully masked so HW output is identical.
SIM_SAFE = False


def _emit_proj(nc, tc, d, consts, ctx, b, a2a_out, chunk=None):
    """Normalize + project one batch's 256-token slice from a2a results.

    chunk=None emits everything; chunk=(k, n) emits part k of n of the
    heavy matmul work (the yt pulls/reciprocal go with chunk 0).
    """
    wp_sb = consts["wp_sb"]
    e16_sb = consts["e16_sb"]
    proj_p, ob_p = ctx["proj_p"], ctx["ob_p"]
    mk_ps = ctx["mk_ps"]          # () -> [P, 512] fp32 PSUM view
    dma_eng = ctx["dma_eng"]      # engine namespace for proj DMAs
    st = ctx.setdefault("state", {})
    key = (b, id(a2a_out[b]))
    if chunk is None or chunk[0] == 0:
        vout = a2a_out[b][:].rearrange("(j h r) t -> h r j t", h=2,
                                       r=DH + 1)
        yt = proj_p.tile([P, DC, 256], BF16, tag="yt", name="yt")
        sums = proj_p.tile([16, 256], BF16, tag="sums", name="sums")
        for h in range(2):
            dma_eng.dma_start(yt[h * DH : (h + 1) * DH], vout[h, 0:DH])
            dma_eng.dma_start(
                sums[h * DC : (h + 1) * DC],
                vout[h, DH : DH + 1].rearrange("r j t -> (r j) t"),
            )
        r16 = proj_p.tile([16, 256], BF16, tag="r16", name="r16")
        with nc.allow_low_precision(
            reason="softmax denominators arrive bf16; bf16 recip ok"
        ):
            nc.vector.reciprocal(r16[:], sums[:])
        ytn = proj_p.tile([P, DC, 256], BF16, tag="ytn", name="ytn")
        for dc in range(DC):
            rb = mk_ps()
            nc.tensor.matmul(
                rb[:, 0:256], e16_sb[:, dc, :], r16[:], start=True, stop=True
            )
            nc.vector.tensor_tensor(ytn[:, dc], yt[:, dc], rb[:, 0:256],
                                    ALU.mult)
        st[key] = ytn
    ytn = st[key]
    pieces = [(tt, hf) for tt in range(2) for hf in range(2)]
    if chunk is not None:
        k, n = chunk
        pieces = pieces[k * len(pieces) // n : (k + 1) * len(pieces) // n]
    for tt, hf in pieces:
        po = mk_ps()
        for dc in range(DC):
            nc.tensor.matmul(
                po[:],
                ytn[:, dc, tt * P : (tt + 1) * P],
                wp_sb[:, dc, hf * 512 : (hf + 1) * 512],
                start=(dc == 0),
                stop=(dc == DC - 1),
            )
        ob = ob_p.tile([P, 512], FP32, tag="ob", name="ob")
        nc.vector.tensor_copy(ob[:], po[:])
        dma_eng.dma_start(
            d["out"][
                b * 256 + tt * P : b * 256 + (tt + 1) * P,
                hf * 512 : (hf + 1) * 512,
            ],
            ob[:],
        )


def _emit_qkv_strip(nc, d, consts, pools, b, s):
    """QKV projection matmuls + drain for one 512-token strip."""
    wqkv_sb = consts["wqkv_sb"]
    pm_p = consts["pm_p"]
    xt, q_sb, k_sb, v_sb = pools["xt"], pools["q_sb"], pools["k_sb"], pools["v_sb"]
    t0 = b * T
    for i, dst in ((0, q_sb), (1, k_sb), (2, v_sb)):
        pm = pm_p.tile([P, 512], FP32, tag="pm")
        for dc in range(DC):
            nc.tensor.matmul(
                pm[:],
                wqkv_sb[:, dc, i * P : (i + 1) * P],
                xt[:, dc, t0 + s * 512 : t0 + (s + 1) * 512],
                start=(dc == 0),
                stop=(dc == DC - 1),
            )
        nc.vector.tensor_copy(dst[:, s * 512 : (s + 1) * 512], pm[:])


def _emit_rope(nc, consts, rope_p, src, dstf):
    """dstf = src*cos + rotate_half(src)*sin on full [128, T] tiles."""
    cos_sb, sin_sb = consts["cos_sb"], consts["sin_sb"]
    perm = rope_p.tile([P, T], BF16, tag="perm")
    for blk in range(4):
        p0 = blk * 32
        src0 = p0 + 32 if blk % 2 == 0 else p0 - 32
        nc.sync.dma_start(perm[p0 : p0 + 32, :], src[src0 : src0 + 32, :])
    nc.vector.tensor_tensor(dstf[:], src[:], cos_sb[:], ALU.mult)
    nc.vector.tensor_tensor(perm[:], perm[:], sin_sb[:], ALU.mult)
    nc.vector.tensor_tensor(dstf[:], dstf[:], perm[:], ALU.add)


def _emit_att_strip(nc, tc, d, consts, pools, b, s, interleave):
    """One attention strip: QK -> exp -> mask -> PV, plus interleaved
    extra PE work (list of thunks) slotted between groups."""
    qk_p, py_p, pt_p, att_p = (consts["qk_p"], consts["py_p"],
                               pools["pt_p"], pools["att_p"])
    qt_f, kt_f = pools["qt_f"], pools["kt_f"]
    va, vb = pools["va"], pools["vb"]
    y2f = pools["y2f"]
    t0 = b * T
    jmax = 4 * s + 3
    pyts = [py_p.tile([DH + 1, 512], FP32, tag="pyt", name=f"pyt{_h}")
            for _h in range(2)]
    n_groups = 2 * s + 2
    for g in range(n_groups):
        for h in range(2):
            hs = slice(h * DH, (h + 1) * DH)
            qkg = qk_p.tile([P, 2, 512], FP32, tag="qkg")
            cw = []
            for m in range(2):
                j = 2 * g + m
                col0 = max(0, P * (j - 4 * s))
                w = 512 - col0
                cw.append((j, col0, w))
                nc.tensor.matmul(
                    qkg[:, m, 0:w],
                    kt_f[hs, j * P : (j + 1) * P],
                    qt_f[hs, s * 512 + col0 : (s + 1) * 512],
                    start=True,
                    stop=True,
                )
            pt = pt_p.tile([P, 2, 512], BF16, tag="pt")
            # Last group of a strip is mostly masked: exp only the
            # live 256 columns.
            ew = 256 if cw[0][1] >= 256 else 512
            if SIM_SAFE and cw[1][2] < ew:
                # The sim refuses to exp uninitialized PSUM; on HW the
                # gap columns are garbage that nothing reads downstream.
                nc.vector.memset(qkg[:, 1, cw[1][2] : ew], -1e4)
            nc.scalar.activation(
                pt[:, :, 0:ew], qkg[:, :, 0:ew], AF.Exp,
                scale=1.0 / math.sqrt(DH),
            )
            tri_sb = consts["tri_sb"]
            for m, (j, col0, w) in enumerate(cw):
                if j >= 4 * s:
                    nc.vector.tensor_tensor(
                        pt[:, m, 0:P], pt[:, m, 0:P], tri_sb[:], ALU.mult
                    )
            v_h = va if h == 0 else vb
            for m, (j, col0, w) in enumerate(cw):
                nc.tensor.matmul(
                    pyts[h][:, col0:512],
                    v_h[:, j, :],
                    pt[:, m, 0:w],
                    start=(j == 0),
                    stop=(j == jmax),
                )
        # Slot one piece of interleaved PE work after each group pair.
        if interleave:
            interleave.pop(0)()
    while interleave:
        interleave.pop(0)()
    sl = slice(s * 512, (s + 1) * 512)
    for h in range(2):
        nc.vector.tensor_copy(y2f[h][:, sl], pyts[h][:])


def _emit_body(nc, tc, d, consts, carry, pending):
    """pending: list of deferred thunks (previous rep's b1 collective).
    Emitted after this rep's b0-phase DMAs so those take descriptor-ring
    slots ahead of the collective and don't chain-wait on it."""
    dram = consts["dram"]

    with (
        tc.tile_pool(name="xt", bufs=1) as xt_p,
        tc.tile_pool(name="qkv", bufs=2) as qkv_p,
        tc.tile_pool(name="rope", bufs=2) as rope_p,
        tc.tile_pool(name="att", bufs=2) as att_p,
        tc.tile_pool(name="pt", bufs=4) as pt_p,
        tc.tile_pool(name="proj", bufs=2) as proj_p,
        tc.tile_pool(name="obp", bufs=2) as ob_p,
    ):
        def mk_pm():
            t = consts["pm_p"].tile([P, 512], FP32, tag="pm", name="pp")
            return t[:]
        ctx_carry = dict(proj_p=proj_p, ob_p=ob_p, mk_ps=mk_pm,
                         dma_eng=nc.gpsimd)
        # ---- x^T streamed in strip-sized slabs (host pre-transposed) ----
        xt = xt_p.tile([P, DC, TOK], BF16, tag="xt")
        xv = d["xt"][:].rearrange("p (dc t) -> p dc t", dc=DC)
        # Prefetch all four two-strip slabs up front; rep i+1's loads
        # overlap rep i's attention tail.
        for b in range(B):
            for s in (0, 2):
                lsl = slice(b * T + s * 512, b * T + (s + 2) * 512)
                nc.sync.dma_start(xt[:, :, lsl], xv[:, :, lsl])

        # Tiles are allocated lazily at first use so the tile-framework
        # reuse guards (which can chain to the previous rep's collective)
        # sit as late as possible in each queue's program order.
        a2a_in, a2a_out = {}, {}

        def mk_a2a(b):
            if b not in a2a_in:
                a2a_in[b] = dram.tile([N_CORES * 2 * (DH + 1), 256], BF16,
                                      tag=f"a2a_in{b}", name=f"a2a_in{b}")
                a2a_out[b] = dram.tile([N_CORES * 2 * (DH + 1), 256], BF16,
                                       tag=f"a2a_out{b}", name=f"a2a_out{b}")

        per_b = {}

        def mk_pools(b):
            if b in per_b:
                return per_b[b]
            q_sb = qkv_p.tile([P, T], BF16, tag="q_sb", name="q_sb")
            k_sb = qkv_p.tile([P, T], BF16, tag="k_sb", name="k_sb")
            v_sb = qkv_p.tile([P, T], BF16, tag="v_sb", name="v_sb")
            qt_f = rope_p.tile([P, T], BF16, tag="qt_f", name="qt_f")
            kt_f = rope_p.tile([P, T], BF16, tag="kt_f", name="kt_f")
            va = att_p.tile([P, T // P, DH + 1], BF16, tag="va", name="va")
            vb = att_p.tile([P, T // P, DH + 1], BF16, tag="vb", name="vb")
            y2f = [att_p.tile([DH + 1, T], BF16, tag=f"y2f{h}",
                              name=f"y2f{h}") for h in range(2)]
            per_b[b] = dict(xt=xt, q_sb=q_sb, k_sb=k_sb, v_sb=v_sb,
                            qt_f=qt_f, kt_f=kt_f, va=va, vb=vb, y2f=y2f,
                            pt_p=pt_p, att_p=att_p)
            return per_b[b]

        def prep_v(b):
            # V into [kv, dh] layout (+ ones column for row sums). PE-mode
            # transposes, NOT dma_start_transpose: the tile framework
            # serializes DMA-transposes against every prior collective
            # (shared XBAR), which was a ~23us rep-boundary stall.
            pools = per_b[b]
            pm_p = consts["pm_p"]
            id_sb = consts["id_sb"]
            for jg in range(4):
                tp = pm_p.tile([P, 4, P], BF16, tag="pm", name="vtp")
                for jj in range(4):
                    j = jg * 4 + jj
                    nc.tensor.transpose(
                        tp[:, jj, :],
                        pools["v_sb"][:, j * P : (j + 1) * P],
                        id_sb[:],
                    )
                for v_h, c0 in ((pools["va"], 0), (pools["vb"], DH)):
                    nc.vector.tensor_copy(
                        v_h[:, jg * 4 : (jg + 1) * 4, 0:DH],
                        tp[:, :, c0 : c0 + DH],
                    )
            nc.vector.memset(pools["va"][:, :, DH], 1.0)
            nc.vector.memset(pools["vb"][:, :, DH], 1.0)

        def stage(b):
            pools = per_b[b]
            mk_a2a(b)
            vin = a2a_in[b][:].rearrange("(j h r) t -> h r j t", h=2,
                                         r=DH + 1)
            with tc.high_priority():
                for h in range(2):
                    yv = pools["y2f"][h][:].rearrange("p (j t) -> p j t",
                                                      j=N_CORES)
                    nc.gpsimd.dma_start(vin[h], yv[:])

        def collect(b):
            nc.gpsimd.collective_compute(
                "AllToAll",
                ALU.bypass,
                replica_groups=[list(range(N_CORES))],
                ins=[a2a_in[b].opt()],
                outs=[a2a_out[b].opt()],
            )

        # ---- batch 0: QKV + rope, serial (PE-bound) ----
        mk_pools(0)
        for s in range(NS):
            _emit_qkv_strip(nc, d, consts, per_b[0], 0, s)
        for src, dstf in ((per_b[0]["q_sb"], per_b[0]["qt_f"]),
                          (per_b[0]["k_sb"], per_b[0]["kt_f"])):
            _emit_rope(nc, consts, rope_p, src, dstf)
        prep_v(0)
        # Previous rep's deferred b1 collective: emitted after this rep's
        # b0-phase DMAs so they precede it in program order and don't
        # inherit a wait on it.
        while pending:
            pending.pop(0)()

        # Carried projections (full rep of slack on their a2a data);
        # slotted into batch-1's attention strips below.
        proj_work = []
        if len(carry) >= 2:
            ao = carry[-2]
            for gb in range(B):
                for k in range(2):
                    proj_work.append(
                        lambda gb=gb, k=k: _emit_proj(
                            nc, tc, d, consts, ctx_carry, gb, ao,
                            chunk=(k, 2)))

        # ---- batch 0 attention, with batch 1's QKV interleaved.
        # Late strips have the most ACT-bound PE slack, so they carry
        # the interleaved work; strip 0 carries none.
        qkv1 = [lambda s=s: _emit_qkv_strip(nc, d, consts, mk_pools(1), 1, s)
                for s in range(NS)]
        slots = [0, 1, 1, 2]
        for s in range(NS):
            inter = [qkv1.pop(0) for _ in range(slots[s]) if qkv1]
            _emit_att_strip(nc, tc, d, consts, per_b[0], 0, s, inter)
        for fn in qkv1:
            fn()
        for src, dstf in ((per_b[1]["q_sb"], per_b[1]["qt_f"]),
                          (per_b[1]["k_sb"], per_b[1]["kt_f"])):
            _emit_rope(nc, consts, rope_p, src, dstf)
        prep_v(1)
        stage(0)
        collect(0)

        # ---- batch 1 attention, with carried projections interleaved ----
        for s in range(NS):
            inter = [proj_work.pop(0) for _ in range(slots[s]) if proj_work]
            _emit_att_strip(nc, tc, d, consts, per_b[1], 1, s, inter)
        for fn in proj_work:
            fn()
        stage(1)
        pending.append(lambda: collect(1))
        return a2a_out


def _build_program(reps=1):
    nc = bacc.Bacc(None, target_bir_lowering=False, debug=False)

    d = {
        "xt": nc.dram_tensor("xt", [P, DC * TOK], BF16, kind="ExternalInput"),
        "wqkv": nc.dram_tensor("wqkv", [P, DC * 3 * P], BF16, kind="ExternalInput"),
        "wp": nc.dram_tensor("wp", [P, DC * D], BF16, kind="ExternalInput"),
        "cos": nc.dram_tensor("cos", [P, T], BF16, kind="ExternalInput"),
        "sin": nc.dram_tensor("sin", [P, T], BF16, kind="ExternalInput"),
        "e16": nc.dram_tensor("e16", [16, DC * P], BF16, kind="ExternalInput"),
        "tri": nc.dram_tensor("tri", [P, P], BF16, kind="ExternalInput"),
        "pmx": nc.dram_tensor("pmx", [P, P], BF16, kind="ExternalInput"),
        "idn": nc.dram_tensor("idn", [P, P], BF16, kind="ExternalInput"),
        "out": nc.dram_tensor("out", [TOK_PER_CORE, D], FP32, kind="ExternalOutput"),
    }

    with tile.TileContext(nc) as tc:
        with (
            tc.tile_pool(name="const", bufs=1) as cpool,
            tc.tile_pool(name="pm_p", bufs=2, space="PSUM") as pm_p,
            tc.tile_pool(name="qk_p", bufs=2, space="PSUM") as qk_p,
            tc.tile_pool(name="py_p", bufs=2, space="PSUM") as py_p,
            tc.tile_pool(name="dram", bufs=8, space="DRAM") as dram,
        ):
            wqkv_sb = cpool.tile([P, DC, 3 * P], BF16)
            nc.sync.dma_start(
                wqkv_sb[:], d["wqkv"][:].rearrange("p (dc c) -> p dc c", dc=DC)
            )
            wp_sb = cpool.tile([P, DC, D], BF16)
            nc.sync.dma_start(
                wp_sb[:], d["wp"][:].rearrange("p (dc c) -> p dc c", dc=DC)
            )
            cos_sb = cpool.tile([P, T], BF16)
            sin_sb = cpool.tile([P, T], BF16)
            nc.sync.dma_start(cos_sb[:], d["cos"][:])
            nc.sync.dma_start(sin_sb[:], d["sin"][:])
            e16_sb = cpool.tile([16, DC, P], BF16)
            nc.sync.dma_start(
                e16_sb[:], d["e16"][:].rearrange("p (dc c) -> p dc c", dc=DC)
            )
            tri_sb = cpool.tile([P, P], BF16)
            nc.sync.dma_start(tri_sb[:], d["tri"][:])
            pmx_sb = cpool.tile([P, P], BF16)
            nc.sync.dma_start(pmx_sb[:], d["pmx"][:])
            id_sb = cpool.tile([P, P], BF16)
            nc.sync.dma_start(id_sb[:], d["idn"][:])

            consts = dict(
                wqkv_sb=wqkv_sb, wp_sb=wp_sb, cos_sb=cos_sb, sin_sb=sin_sb,
                e16_sb=e16_sb, tri_sb=tri_sb, pmx_sb=pmx_sb, id_sb=id_sb,
                dram=dram, pm_p=pm_p, qk_p=qk_p, py_p=py_p,
            )
            carry = []
            pending = []
            for _rep in range(reps):
                carry.append(_emit_body(nc, tc, d, consts, carry, pending))
            while pending:
                pending.pop(0)()
            with tc.tile_pool(name="projf", bufs=1) as proj_f, \
                 tc.tile_pool(name="obf", bufs=1) as ob_f:
                def mk_pmf():
                    t = pm_p.tile([P, 512], FP32, tag="pm", name="pf")
                    return t[:]
                ctx_f = dict(proj_p=proj_f, ob_p=ob_f, mk_ps=mk_pmf,
                             dma_eng=nc.gpsimd)
                for ao in carry[-2:] if len(carry) >= 2 else carry:
                    for gb in range(B):
                        _emit_proj(nc, tc, d, consts, ctx_f, gb, ao)

    nc.compile()
    return nc


_NC_CACHE = {}


def _get_program(reps=1):
    if reps not in _NC_CACHE:
        _NC_CACHE[reps] = _build_program(reps)
    return _NC_CACHE[reps]


def _host_tables():
    inv_freq = 1.0 / (ROPE_BASE ** (np.arange(0, DH, 2, dtype=np.float32) / DH))
    t = np.arange(T, dtype=np.float32)
    freqs = np.outer(t, inv_freq).astype(np.float32)  # (T, 32)
    cos_t = np.cos(freqs).T                           # (32, T)
    sin_t = np.sin(freqs).T
    cos = np.empty((P, T), np.float32)
    sin = np.empty((P, T), np.float32)
    for blk in range(4):
        cos[blk * 32 : (blk + 1) * 32] = cos_t
        sgn = -1.0 if blk % 2 == 0 else 1.0
        sin[blk * 32 : (blk + 1) * 32] = sgn * sin_t
    return cos, sin


def make_in_maps(x, W_qkv, W_proj):
    bf = ml_dtypes.bfloat16
    x = np.asarray(x, np.float32).reshape(TOK, D)
    # xt[p, dc*TOK + t] = x[t, dc*P + p]
    xt = np.ascontiguousarray(
        x.T.reshape(DC, P, TOK).transpose(1, 0, 2).reshape(P, DC * TOK)
    ).astype(bf)
    W_qkv = np.asarray(W_qkv, np.float32)
    W_proj = np.asarray(W_proj, np.float32)
    cos, sin = _host_tables()
    cosb, sinb = cos.astype(bf), sin.astype(bf)

    # wp[p, dc*D + oc] = W_proj[dc*P + p, oc]
    wp = np.ascontiguousarray(
        W_proj.reshape(DC, P, D).transpose(1, 0, 2).reshape(P, DC * D)
    ).astype(bf)

    e16 = np.zeros((16, DC, P), np.float32)
    for dc in range(DC):
        for p in range(P):
            e16[(p // DH) * DC + dc, dc, p] = 1.0
    e16 = e16.reshape(16, DC * P).astype(bf)
    # tri[p, f] = 1 where f >= p: keeps q-column f for kv row p.
    tri = np.triu(np.ones((P, P), np.float32)).astype(bf)
    # pmx[k, m] = 1 iff k == swap(m): rotate-half 32-row block swap.
    pmx = np.zeros((P, P), np.float32)
    for m in range(P):
        sw = m + 32 if (m // 32) % 2 == 0 else m - 32
        pmx[sw, m] = 1.0
    pmx = pmx.astype(bf)
    idn = np.eye(P, dtype=np.float32).astype(bf)

    in_maps = []
    for c in range(N_CORES):
        # wqkv[p, dc, i*P + j] = W_qkv[dc*P + p, i*D + c*P + j]
        wq = np.empty((P, DC, 3 * P), np.float32)
        for i in range(3):
            blk = W_qkv[:, i * D + c * P : i * D + (c + 1) * P]  # [D, P]
            wq[:, :, i * P : (i + 1) * P] = blk.reshape(DC, P, P).transpose(1, 0, 2)
        in_maps.append(
            {
                "xt": xt,
                "wqkv": np.ascontiguousarray(wq.reshape(P, DC * 3 * P)).astype(bf),
                "wp": wp,
                "cos": cosb,
                "sin": sinb,
                "e16": e16,
                "tri": tri,
                "pmx": pmx,
                "idn": idn,
            }
        )
    return in_maps


def kernel(x, W_qkv, W_proj):
    in_maps = make_in_maps(x, W_qkv, W_proj)
    nc = _get_program()
    res = run_bass_kernel_spmd(nc, in_maps, list(range(N_CORES)))
    return assemble([res.results[c]["out"] for c in range(N_CORES)])


def assemble(outs):
    full = np.empty((B, T, D), np.float32)
    for c in range(N_CORES):
        o = outs[c]
        for b in range(B):
            full[b, 256 * c : 256 * (c + 1)] = o[b * 256 : (b + 1) * 256]
    return full
